# revision 4
# baseline (speedup 1.0000x reference)
"""Trainium2 Bass kernel for nn_LowRankLayer_dilation (B=4, C=64, H=W=128).

Math: the reference's rank-3 NMF update collapses exactly (all ranks are
initialized identically), and the eps terms are negligible for this input
distribution, giving:

    h    = relu(W_head @ x)           (per-pixel channel matmul)
    av   = box9(h)                    (3x3 dilation-2 box sum, edge-clamped)
    n'_k = sum_c av_c * h_c(p+d_k)    (9 taps, d in {-2,0,2}^2; n' = 9^2 n)
    q    = sum_k n'_k * h(p+d_k)
    out  = x + (n'_4 / sum_j n'_j^2) * (W_tail @ q)

(All 9/81 scale factors cancel between n'_4/sum n'^2 and q.)

vs the first-pass kernel: the W_tail matmul is folded into the per-k facc
accumulation (no g tensor at all), the f32 residual load is dropped (the
bf16 x already on chip serves as residual), the output is bf16 (host
upcasts), matmuls are 1024-wide, and the elementwise work is spread across
vector + scalar + gpsimd.

Sharding: pure data parallel, 8 cores = (batch b, H-half). Each core gets a
68-row halo'd slice packed as 2 channel blocks on 128 partitions:
partition p = c + 64*blk, blk A = slice rows 0..35, blk B = rows 32..67.
Channel reductions/broadcasts run on the PE via block-structured 0/1
matrices. h is stored with 2 replicate-padded columns on each side
(row stride 132), so every dilated tap is a pure strided AP view.
"""
import sys
import contextlib
import numpy as np

sys.path.insert(0, '/opt/trn_rl_repo')

import concourse.bass as bass  # noqa: E402,F401
import concourse.bacc as bacc  # noqa: E402
import concourse.tile as tile  # noqa: E402
import concourse.mybir as mybir  # noqa: E402
from concourse.bass_utils import run_bass_kernel_spmd  # noqa: E402

F32 = mybir.dt.float32
BF16 = mybir.dt.bfloat16
AT = mybir.ActivationFunctionType
OP = mybir.AluOpType

N_CORES = 8
RIN = 36          # per-block input rows (with +-2 halo)
ROUT = 32         # per-block output rows
W = 128
WP = W + 4        # padded row stride for h
FIN = RIN * W     # 4608
FOUT = ROUT * W   # 4096
HF = 2048         # half of FOUT
OFFS = [(di, dj) for di in (-2, 0, 2) for dj in (-2, 0, 2)]

GP_PK = (2, 6)    # k's whose nb*h_tap mul runs on gpsimd (per half)
XRES0 = 2 * W     # xbt column where the residual (non-halo) rows start


def _build():
    nc = bacc.Bacc("TRN2", target_bir_lowering=False, debug=False,
                   num_devices=N_CORES)
    xb_ext = nc.dram_tensor("xb", [128, FIN], BF16, kind="ExternalInput").ap()
    w2_ext = nc.dram_tensor("w2", [128, 128], BF16, kind="ExternalInput").ap()
    w3_ext = nc.dram_tensor("w3", [128, 128], BF16, kind="ExternalInput").ap()
    bo_ext = nc.dram_tensor("bo", [128, 128], BF16, kind="ExternalInput").ap()
    sb_ext = nc.dram_tensor("sb", [18, 2], BF16, kind="ExternalInput").ap()
    bc2_ext = nc.dram_tensor("bc2", [2, 128], BF16, kind="ExternalInput").ap()
    y_ext = nc.dram_tensor("y", [128, FOUT], BF16, kind="ExternalOutput").ap()

    with tile.TileContext(nc) as tc, contextlib.ExitStack() as ctx:
        cpool = ctx.enter_context(tc.tile_pool(name="consts", bufs=1))
        big = ctx.enter_context(tc.tile_pool(name="big", bufs=1))
        ppool = ctx.enter_context(tc.tile_pool(name="prod", bufs=5))
        npool = ctx.enter_context(tc.tile_pool(name="nbuf", bufs=4))

        xbt = big.tile([128, FIN], BF16)
        for c in range(3):
            nc.sync.dma_start(xbt[:, c * 1536:(c + 1) * 1536],
                              xb_ext[:, c * 1536:(c + 1) * 1536])
        w2 = cpool.tile([128, 128], BF16)
        nc.sync.dma_start(w2[:], w2_ext[:])
        w3 = cpool.tile([128, 128], BF16)
        nc.sync.dma_start(w3[:], w3_ext[:])
        bo = cpool.tile([128, 128], BF16)
        nc.gpsimd.dma_start(bo[:], bo_ext[:])
        sbm = cpool.tile([18, 2], BF16)
        nc.gpsimd.dma_start(sbm[:], sb_ext[:])
        bc2 = cpool.tile([2, 128], BF16)
        nc.gpsimd.dma_start(bc2[:], bc2_ext[:])

        # h: (RIN, WP) row layout; data at cols 2..129, replicate pads at
        # cols 0,1,130,131. A (di,dj) tap over rows is then a pure strided
        # 3D view with the edge clamp built in.
        hf = big.tile([128, RIN * WP], BF16)
        h3 = hf.rearrange("p (r w) -> p r w", w=WP)

        def tap(t3, di, dj, rows=ROUT, r0=2):
            rr = r0 + di
            return t3[:, rr:rr + rows, 2 + dj:2 + dj + W]

        T = big.tile([128, FIN], BF16)
        T3 = T.rearrange("p (r w) -> p r w", w=W)
        av = big.tile([128, FOUT], BF16)
        av3 = av.rearrange("p (r w) -> p r w", w=W)

        # ---- head matmul h = relu(W_head @ x), 1024-wide, relu split
        # scalar/vector; per-chunk pads + column-sum T chunk chase each
        # relu chunk so the box filter overlaps the head matmuls ----
        with tc.tile_pool(name="psmm", bufs=2, space="PSUM") as psmm:

            def pads_T(r0, nr):
                for dst, src in ((0, 2), (1, 2), (130, 129), (131, 129)):
                    nc.vector.tensor_copy(h3[:, r0:r0 + nr, dst:dst + 1],
                                          h3[:, r0:r0 + nr, src:src + 1])
                nc.vector.tensor_add(T3[:, r0:r0 + nr, :],
                                     tap(h3, -2, -2, nr, 2 + r0),
                                     tap(h3, -2, 0, nr, 2 + r0))
                nc.vector.tensor_add(T3[:, r0:r0 + nr, :],
                                     T3[:, r0:r0 + nr, :],
                                     tap(h3, -2, 2, nr, 2 + r0))

            for j in range(4):
                ps = psmm.tile([128, 1024], F32)
                for q in range(2):
                    c0 = j * 1024 + q * 512
                    nc.tensor.matmul(ps[:, q * 512:(q + 1) * 512], w2[:],
                                     xbt[:, c0:c0 + 512], start=True, stop=True)
                r0 = j * 8
                hv = h3[:, r0:r0 + 8, 2:2 + W]
                pv = ps[:].rearrange("p (r w) -> p r w", w=W)
                if j % 2 == 0:
                    nc.scalar.activation(hv, pv, AT.Relu)
                else:
                    nc.vector.tensor_relu(hv, pv)
                pads_T(r0, 8)
            ps = psmm.tile([128, 1024], F32)
            nc.tensor.matmul(ps[:, 0:512], w2[:], xbt[:, 4096:4608],
                             start=True, stop=True)
            nc.scalar.activation(h3[:, 32:36, 2:2 + W],
                                 ps[:, 0:512].rearrange("p (r w) -> p r w", w=W),
                                 AT.Relu)
            pads_T(32, 4)
            # row 3-tap: av[r] = T[r] + T[r+2] + T[r+4], per half
            for hh in range(2):
                s = hh * HF
                nc.vector.tensor_add(av[:, s:s + HF], T[:, s:s + HF],
                                     T[:, s + 2 * W:s + 2 * W + HF])
                nc.vector.tensor_add(av[:, s:s + HF], av[:, s:s + HF],
                                     T[:, s + 4 * W:s + 4 * W + HF])

        # ---- per-k: n'_k (PE reduce+broadcast), facc = sum_k W_tail@(n'_k
        # h_tap) accumulated on the PE. Two half-passes (16 out-rows each).
        # The Cf / output chain of each half is emitted lagged, inside the
        # next half's k-loop. ----
        nst = cpool.tile([18, FOUT], BF16)      # n' rows, row pair by kr
        nsq = cpool.tile([18, FOUT], BF16)
        facc = big.tile([128, FOUT], BF16)
        cfr = cpool.tile([2, FOUT], BF16)

        with tc.tile_pool(name="psnk", bufs=2, space="PSUM") as psnk, \
                tc.tile_pool(name="psfa", bufs=1, space="PSUM") as psfa, \
                tc.tile_pool(name="rows", bufs=1) as rows:

            def cf_steps(half):
                """Deferred tail for one half: Cf row computation, cfb
                broadcast, residual, DMA out."""
                hs = slice(half * HF, (half + 1) * HF)
                nc.scalar.activation(nsq[:, hs], nst[:, hs], AT.Square)
                s2ps = psfa.tile([2, HF], F32, tag="facc_ps")
                for q in range(4):
                    c0 = half * HF + q * 512
                    nc.tensor.matmul(s2ps[:, q * 512:(q + 1) * 512], sbm[:],
                                     nsq[:, c0:c0 + 512],
                                     start=True, stop=True)
                rcp = rows.tile([2, HF], F32, tag="rcp")
                nc.vector.reciprocal_approx_fast(rcp[:], s2ps[:])
                nc.vector.tensor_mul(cfr[:, hs], nst[0:2, hs], rcp[:])
                yield
                for ch in range(2):
                    sl = slice(half * HF + ch * 1024,
                               half * HF + (ch + 1) * 1024)
                    cfb = psnk.tile([128, 1024], F32, tag="nk")
                    for q in range(2):
                        c0 = half * HF + ch * 1024 + q * 512
                        nc.tensor.matmul(cfb[:, q * 512:(q + 1) * 512], bc2[:],
                                         cfr[:, c0:c0 + 512],
                                         start=True, stop=True)
                    resm = npool.tile([128, 1024], BF16, tag="res")
                    nc.vector.tensor_mul(resm[:], facc[:, sl], cfb[:])
                    ysb = npool.tile([128, 1024], BF16, tag="ysb")
                    xres = xbt[:, XRES0 + half * HF + ch * 1024:
                               XRES0 + half * HF + (ch + 1) * 1024]
                    if half == 1:
                        nc.gpsimd.tensor_add(ysb[:], resm[:], xres)
                    else:
                        nc.vector.tensor_add(ysb[:], resm[:], xres)
                    nc.gpsimd.dma_start(y_ext[:, sl], ysb[:])
                    yield

            pending = None                    # deferred cf-chain generator
            for half in range(2):
                rh = half * 16

                def emit_prod(k):
                    di, dj = OFFS[k]
                    prod = ppool.tile([128, HF], BF16, tag="pp")
                    p3 = prod.rearrange("p (r w) -> p r w", w=W)
                    nc.vector.tensor_mul(
                        p3[:], av3[:, rh:rh + 16, :],
                        tap(h3, di, dj, rows=16, r0=2 + rh))
                    return prod

                prods = {0: emit_prod(0)}
                facc_ps = psfa.tile([128, HF], F32, tag="facc_ps")
                pks = {}

                def emit_facc(k):
                    pk = pks.pop(k)
                    for q in range(4):
                        c0 = q * 512
                        nc.tensor.matmul(facc_ps[:, c0:c0 + 512], w3[:],
                                         pk[:, c0:c0 + 512],
                                         start=(k == 0), stop=(k == 8))

                for k, (di, dj) in enumerate(OFFS):
                    prod = prods.pop(k)
                    nb = npool.tile([128, HF], BF16, tag="nb")
                    for q in range(2):
                        pst = psnk.tile([128, 1024], F32, tag="nk")
                        for u in range(2):
                            c0 = q * 1024 + u * 512
                            nc.tensor.matmul(pst[:, u * 512:(u + 1) * 512],
                                             bo[:], prod[:, c0:c0 + 512],
                                             start=True, stop=True)
                        nc.scalar.copy(nb[:, q * 1024:(q + 1) * 1024], pst[:])
                    kr = (k - 4) % 9          # put k=4 (center) at rows 0..1
                    hs = slice(half * HF, (half + 1) * HF)
                    nc.sync.dma_start(nst[2 * kr:2 * kr + 2, hs],
                                      nb[0:128:64, :])

                    if k + 1 < 9:
                        prods[k + 1] = emit_prod(k + 1)

                    nb3 = nb.rearrange("p (r w) -> p r w", w=W)
                    pk = ppool.tile([128, HF], BF16, tag="pp")
                    p3 = pk.rearrange("p (r w) -> p r w", w=W)
                    eng = nc.gpsimd if k in GP_PK else nc.vector
                    eng.tensor_mul(p3[:], nb3[:],
                                   tap(h3, di, dj, rows=16, r0=2 + rh))
                    pks[k] = pk
                    if k >= 1:
                        emit_facc(k - 1)
                    if pending is not None and k in (5, 7):
                        next(pending, None)
                emit_facc(8)
                nc.scalar.copy(facc[:, half * HF:(half + 1) * HF], facc_ps[:])
                if pending is not None:
                    for _ in pending:
                        pass
                pending = cf_steps(half)
                next(pending, None)   # emit Cf-row chain right away
            for _ in pending:
                pass

    nc.compile()
    return nc


_NC_CACHE = [None]


def _get_nc():
    if _NC_CACHE[0] is None:
        _NC_CACHE[0] = _build()
    return _NC_CACHE[0]


def _host_prep(x):
    import ml_dtypes
    B, Cc, H, Ww = x.shape
    in_maps = []
    for core in range(N_CORES):
        b, half = core // 2, core % 2
        r0 = 64 * half
        gidx = np.clip(np.arange(r0 - 2, r0 + 66), 0, H - 1)
        xs = x[b][:, gidx, :]                     # (64, 68, 128)
        packed = np.ascontiguousarray(
            np.concatenate([xs[:, 0:36], xs[:, 32:68]], axis=0))
        in_maps.append({
            "xb": packed.reshape(128, FIN).astype(ml_dtypes.bfloat16),
        })
    return in_maps


def _const_maps(W_head, W_tail):
    import ml_dtypes

    def to_bf(a):
        return a.astype(ml_dtypes.bfloat16)

    w2 = np.zeros((128, 128), np.float32)
    w2[:64, :64] = W_head.T
    w2[64:, 64:] = W_head.T
    w3 = np.zeros((128, 128), np.float32)
    w3[:64, :64] = W_tail.T
    w3[64:, 64:] = W_tail.T
    bo = np.zeros((128, 128), np.float32)
    bo[:64, :64] = 1.0
    bo[64:, 64:] = 1.0
    sb = np.zeros((18, 2), np.float32)
    sb[0::2, 0] = 1.0
    sb[1::2, 1] = 1.0
    bc2 = np.zeros((2, 128), np.float32)
    bc2[0, :64] = 1.0
    bc2[1, 64:] = 1.0
    return {"w2": to_bf(w2), "w3": to_bf(w3), "bo": to_bf(bo),
            "sb": to_bf(sb), "bc2": to_bf(bc2)}


def kernel(x, W_head, W_tail):
    x = np.asarray(x, np.float32)
    W_head = np.asarray(W_head, np.float32)
    W_tail = np.asarray(W_tail, np.float32)
    nc = _get_nc()
    consts = _const_maps(W_head, W_tail)
    in_maps = [{**m, **consts} for m in _host_prep(x)]
    res = run_bass_kernel_spmd(nc, in_maps, list(range(N_CORES)))
    out = np.empty_like(x)
    for core in range(N_CORES):
        b, half = core // 2, core % 2
        r0 = 64 * half
        y = res.results[core]["y"].astype(np.float32).reshape(128, ROUT, W)
        out[b, :, r0:r0 + 32, :] = y[:64]
        out[b, :, r0 + 32:r0 + 64, :] = y[64:]
    return out


# revision 6
# speedup vs baseline: 1.0703x; 1.0703x over previous
"""Trainium2 Bass kernel for nn_LowRankLayer_dilation (B=4, C=64, H=W=128).

Math: the reference's rank-3 NMF update collapses exactly (all ranks are
initialized identically), and the eps terms are negligible for this input
distribution, giving:

    h    = relu(W_head @ x)           (per-pixel channel matmul)
    av   = box9(h)                    (3x3 dilation-2 box sum, edge-clamped)
    n'_k = sum_c av_c * h_c(p+d_k)    (9 taps, d in {-2,0,2}^2; n' = 9^2 n)
    q    = sum_k n'_k * h(p+d_k)
    out  = x + (n'_4 / sum_j n'_j^2) * (W_tail @ q)

(All 9/81 scale factors cancel between n'_4/sum n'^2 and q.)

vs the first-pass kernel: the W_tail matmul is folded into the per-k facc
accumulation (no g tensor at all), the f32 residual load is dropped (the
bf16 x already on chip serves as residual), the output is bf16 (host
upcasts), matmuls are 1024-wide, and the elementwise work is spread across
vector + scalar + gpsimd.

Sharding: pure data parallel, 8 cores = (batch b, H-half). Each core gets a
68-row halo'd slice packed as 2 channel blocks on 128 partitions:
partition p = c + 64*blk, blk A = slice rows 0..35, blk B = rows 32..67.
Channel reductions/broadcasts run on the PE via block-structured 0/1
matrices. h is stored with 2 replicate-padded columns on each side
(row stride 132), so every dilated tap is a pure strided AP view.
"""
import sys
import contextlib
import numpy as np

sys.path.insert(0, '/opt/trn_rl_repo')

import concourse.bass as bass  # noqa: E402,F401
import concourse.bacc as bacc  # noqa: E402
import concourse.tile as tile  # noqa: E402
import concourse.mybir as mybir  # noqa: E402
from concourse.bass_utils import run_bass_kernel_spmd  # noqa: E402

F32 = mybir.dt.float32
BF16 = mybir.dt.bfloat16
AT = mybir.ActivationFunctionType
OP = mybir.AluOpType

N_CORES = 8
RIN = 36          # per-block input rows (with +-2 halo)
ROUT = 32         # per-block output rows
W = 128
WP = W + 4        # padded row stride for h
FIN = RIN * W     # 4608
FOUT = ROUT * W   # 4096
HF = 2048         # half of FOUT
OFFS = [(di, dj) for di in (-2, 0, 2) for dj in (-2, 0, 2)]

GP_PK = ()        # k's whose nb*h_tap mul runs on gpsimd (per half)
XRES0 = 2 * W     # xbt column where the residual (non-halo) rows start


def _build():
    nc = bacc.Bacc("TRN2", target_bir_lowering=False, debug=False,
                   num_devices=N_CORES)
    xb_ext = nc.dram_tensor("xb", [128, FIN], BF16, kind="ExternalInput").ap()
    w2_ext = nc.dram_tensor("w2", [128, 128], BF16, kind="ExternalInput").ap()
    w3_ext = nc.dram_tensor("w3", [128, 128], BF16, kind="ExternalInput").ap()
    bo_ext = nc.dram_tensor("bo", [128, 128], BF16, kind="ExternalInput").ap()
    sb_ext = nc.dram_tensor("sb", [18, 2], BF16, kind="ExternalInput").ap()
    bc2_ext = nc.dram_tensor("bc2", [2, 128], BF16, kind="ExternalInput").ap()
    y_ext = nc.dram_tensor("y", [128, FOUT], BF16, kind="ExternalOutput").ap()

    with tile.TileContext(nc) as tc, contextlib.ExitStack() as ctx:
        cpool = ctx.enter_context(tc.tile_pool(name="consts", bufs=1))
        big = ctx.enter_context(tc.tile_pool(name="big", bufs=1))
        ppool = ctx.enter_context(tc.tile_pool(name="prod", bufs=5))
        npool = ctx.enter_context(tc.tile_pool(name="nbuf", bufs=4))

        w2 = cpool.tile([128, 128], BF16)
        nc.sync.dma_start(w2[:], w2_ext[:])
        xbt = big.tile([128, FIN], BF16)
        nc.sync.dma_start(xbt[:, 0:512], xb_ext[:, 0:512])
        for c in range(4):
            nc.sync.dma_start(xbt[:, 512 + c * 1024:512 + (c + 1) * 1024],
                              xb_ext[:, 512 + c * 1024:512 + (c + 1) * 1024])
        w3 = cpool.tile([128, 128], BF16)
        nc.sync.dma_start(w3[:], w3_ext[:])
        bo = cpool.tile([128, 128], BF16)
        nc.gpsimd.dma_start(bo[:], bo_ext[:])
        sbm = cpool.tile([18, 2], BF16)
        nc.gpsimd.dma_start(sbm[:], sb_ext[:])
        bc2 = cpool.tile([2, 128], BF16)
        nc.gpsimd.dma_start(bc2[:], bc2_ext[:])

        # h: (RIN, WP) row layout; data at cols 2..129, replicate pads at
        # cols 0,1,130,131. A (di,dj) tap over rows is then a pure strided
        # 3D view with the edge clamp built in.
        hf = big.tile([128, RIN * WP], BF16)
        h3 = hf.rearrange("p (r w) -> p r w", w=WP)

        def tap(t3, di, dj, rows=ROUT, r0=2):
            rr = r0 + di
            return t3[:, rr:rr + rows, 2 + dj:2 + dj + W]

        T = big.tile([128, FIN], BF16)
        T3 = T.rearrange("p (r w) -> p r w", w=W)
        av = big.tile([128, FOUT], BF16)
        av3 = av.rearrange("p (r w) -> p r w", w=W)

        # ---- PE warm-up: ~5us of dummy matmuls during the input DMAs so
        # the HAM clock-gate opens (1.2 -> 2.4 GHz) before real work ----
        wsc = cpool.tile([128, 640], BF16)
        nc.vector.memset(wsc[:], 0.0)
        with tc.tile_pool(name="pswarm", bufs=1, space="PSUM") as pswarm:
            wps = pswarm.tile([128, 512], F32)
            for _ in range(14):
                nc.tensor.matmul(wps[:], wsc[:, 0:128], wsc[:, 128:640],
                                 start=True, stop=True)

        # ---- head matmul h = relu(W_head @ x), 1024-wide, relu split
        # scalar/vector; per-chunk pads + column-sum T chunk chase each
        # relu chunk so the box filter overlaps the head matmuls ----
        with tc.tile_pool(name="psmm", bufs=2, space="PSUM") as psmm:

            def pads_T(r0, nr):
                for dst, src in ((0, 2), (1, 2), (130, 129), (131, 129)):
                    nc.vector.tensor_copy(h3[:, r0:r0 + nr, dst:dst + 1],
                                          h3[:, r0:r0 + nr, src:src + 1])
                nc.vector.tensor_add(T3[:, r0:r0 + nr, :],
                                     tap(h3, -2, -2, nr, 2 + r0),
                                     tap(h3, -2, 0, nr, 2 + r0))
                nc.vector.tensor_add(T3[:, r0:r0 + nr, :],
                                     T3[:, r0:r0 + nr, :],
                                     tap(h3, -2, 2, nr, 2 + r0))

            for j in range(4):
                ps = psmm.tile([128, 1024], F32)
                for q in range(2):
                    c0 = j * 1024 + q * 512
                    nc.tensor.matmul(ps[:, q * 512:(q + 1) * 512], w2[:],
                                     xbt[:, c0:c0 + 512], start=True, stop=True)
                r0 = j * 8
                hv = h3[:, r0:r0 + 8, 2:2 + W]
                pv = ps[:].rearrange("p (r w) -> p r w", w=W)
                if j % 2 == 0:
                    nc.scalar.activation(hv, pv, AT.Relu)
                else:
                    nc.vector.tensor_relu(hv, pv)
                pads_T(r0, 8)
            ps = psmm.tile([128, 1024], F32)
            nc.tensor.matmul(ps[:, 0:512], w2[:], xbt[:, 4096:4608],
                             start=True, stop=True)
            nc.scalar.activation(h3[:, 32:36, 2:2 + W],
                                 ps[:, 0:512].rearrange("p (r w) -> p r w", w=W),
                                 AT.Relu)
            pads_T(32, 4)
            # row 3-tap: av[r] = T[r] + T[r+2] + T[r+4], per half
            for hh in range(2):
                s = hh * HF
                nc.vector.tensor_add(av[:, s:s + HF], T[:, s:s + HF],
                                     T[:, s + 2 * W:s + 2 * W + HF])
                nc.vector.tensor_add(av[:, s:s + HF], av[:, s:s + HF],
                                     T[:, s + 4 * W:s + 4 * W + HF])

        # ---- per-k: n'_k (PE reduce+broadcast), facc = sum_k W_tail@(n'_k
        # h_tap) accumulated on the PE. Two half-passes (16 out-rows each).
        # The Cf / output chain of each half is emitted lagged, inside the
        # next half's k-loop. ----
        nst = cpool.tile([18, FOUT], BF16)      # n' rows, row pair by kr
        nsq = cpool.tile([18, FOUT], BF16)
        facc = big.tile([128, FOUT], BF16)
        cfr = cpool.tile([2, FOUT], BF16)

        with tc.tile_pool(name="psnk", bufs=2, space="PSUM") as psnk, \
                tc.tile_pool(name="psfa", bufs=1, space="PSUM") as psfa, \
                tc.tile_pool(name="rows", bufs=1) as rows:

            def cf_steps(half):
                """Deferred tail for one half: Cf row computation, cfb
                broadcast, residual, DMA out."""
                hs = slice(half * HF, (half + 1) * HF)
                s2ps = psfa.tile([2, HF], F32, tag="facc_ps")
                for q in range(4):
                    c0 = half * HF + q * 512
                    nc.tensor.matmul(s2ps[:, q * 512:(q + 1) * 512], sbm[:],
                                     nsq[:, c0:c0 + 512],
                                     start=True, stop=True)
                rcp = rows.tile([2, HF], F32, tag="rcp")
                nc.vector.reciprocal_approx_fast(rcp[:], s2ps[:])
                nc.vector.tensor_mul(cfr[:, hs], nst[0:2, hs], rcp[:])
                yield
                for ch in range(2):
                    sl = slice(half * HF + ch * 1024,
                               half * HF + (ch + 1) * 1024)
                    cfb = psnk.tile([128, 1024], F32, tag="nk")
                    for q in range(2):
                        c0 = half * HF + ch * 1024 + q * 512
                        nc.tensor.matmul(cfb[:, q * 512:(q + 1) * 512], bc2[:],
                                         cfr[:, c0:c0 + 512],
                                         start=True, stop=True)
                    resm = npool.tile([128, 1024], BF16, tag="res")
                    nc.vector.tensor_mul(resm[:], facc[:, sl], cfb[:])
                    ysb = npool.tile([128, 1024], BF16, tag="ysb")
                    xres = xbt[:, XRES0 + half * HF + ch * 1024:
                               XRES0 + half * HF + (ch + 1) * 1024]
                    if half == 0:
                        nc.gpsimd.tensor_add(ysb[:], resm[:], xres)
                    else:
                        nc.vector.tensor_add(ysb[:], resm[:], xres)
                    nc.gpsimd.dma_start(y_ext[:, sl], ysb[:])
                    yield

            pending = None                    # deferred cf-chain generator
            for half in range(2):
                rh = half * 16

                def emit_prod(k):
                    di, dj = OFFS[k]
                    prod = ppool.tile([128, HF], BF16, tag="pp")
                    p3 = prod.rearrange("p (r w) -> p r w", w=W)
                    nc.vector.tensor_mul(
                        p3[:], av3[:, rh:rh + 16, :],
                        tap(h3, di, dj, rows=16, r0=2 + rh))
                    return prod

                prods = {0: emit_prod(0)}
                facc_ps = psfa.tile([128, HF], F32, tag="facc_ps")
                pks = {}

                def emit_facc(k):
                    pk = pks.pop(k)
                    for q in range(4):
                        c0 = q * 512
                        nc.tensor.matmul(facc_ps[:, c0:c0 + 512], w3[:],
                                         pk[:, c0:c0 + 512],
                                         start=(k == 0), stop=(k == 8))

                for k, (di, dj) in enumerate(OFFS):
                    prod = prods.pop(k)
                    nb = npool.tile([128, HF], BF16, tag="nb")
                    for q in range(2):
                        pst = psnk.tile([128, 1024], F32, tag="nk")
                        for u in range(2):
                            c0 = q * 1024 + u * 512
                            nc.tensor.matmul(pst[:, u * 512:(u + 1) * 512],
                                             bo[:], prod[:, c0:c0 + 512],
                                             start=True, stop=True)
                        nc.scalar.copy(nb[:, q * 1024:(q + 1) * 1024], pst[:])
                    kr = (k - 4) % 9          # put k=4 (center) at rows 0..1
                    hs = slice(half * HF, (half + 1) * HF)
                    nc.sync.dma_start(nst[2 * kr:2 * kr + 2, hs],
                                      nb[0:128:64, :])

                    if k + 1 < 9:
                        prods[k + 1] = emit_prod(k + 1)

                    nb3 = nb.rearrange("p (r w) -> p r w", w=W)
                    pk = ppool.tile([128, HF], BF16, tag="pp")
                    p3 = pk.rearrange("p (r w) -> p r w", w=W)
                    eng = nc.gpsimd if k in GP_PK else nc.vector
                    eng.tensor_mul(p3[:], nb3[:],
                                   tap(h3, di, dj, rows=16, r0=2 + rh))
                    pks[k] = pk
                    if k >= 1:
                        emit_facc(k - 1)
                    if pending is not None and k in (5, 7):
                        next(pending, None)
                emit_facc(8)
                hs8 = slice(half * HF, (half + 1) * HF)
                nc.scalar.activation(nsq[:, hs8], nst[:, hs8], AT.Square)
                nc.scalar.copy(facc[:, half * HF:(half + 1) * HF], facc_ps[:])
                if pending is not None:
                    for _ in pending:
                        pass
                pending = cf_steps(half)
                next(pending, None)   # emit Cf-row chain right away
            for _ in pending:
                pass

    nc.compile()
    return nc


_NC_CACHE = [None]


def _get_nc():
    if _NC_CACHE[0] is None:
        _NC_CACHE[0] = _build()
    return _NC_CACHE[0]


def _host_prep(x):
    import ml_dtypes
    B, Cc, H, Ww = x.shape
    in_maps = []
    for core in range(N_CORES):
        b, half = core // 2, core % 2
        r0 = 64 * half
        gidx = np.clip(np.arange(r0 - 2, r0 + 66), 0, H - 1)
        xs = x[b][:, gidx, :]                     # (64, 68, 128)
        packed = np.ascontiguousarray(
            np.concatenate([xs[:, 0:36], xs[:, 32:68]], axis=0))
        in_maps.append({
            "xb": packed.reshape(128, FIN).astype(ml_dtypes.bfloat16),
        })
    return in_maps


def _const_maps(W_head, W_tail):
    import ml_dtypes

    def to_bf(a):
        return a.astype(ml_dtypes.bfloat16)

    w2 = np.zeros((128, 128), np.float32)
    w2[:64, :64] = W_head.T
    w2[64:, 64:] = W_head.T
    w3 = np.zeros((128, 128), np.float32)
    w3[:64, :64] = W_tail.T
    w3[64:, 64:] = W_tail.T
    bo = np.zeros((128, 128), np.float32)
    bo[:64, :64] = 1.0
    bo[64:, 64:] = 1.0
    sb = np.zeros((18, 2), np.float32)
    sb[0::2, 0] = 1.0
    sb[1::2, 1] = 1.0
    bc2 = np.zeros((2, 128), np.float32)
    bc2[0, :64] = 1.0
    bc2[1, 64:] = 1.0
    return {"w2": to_bf(w2), "w3": to_bf(w3), "bo": to_bf(bo),
            "sb": to_bf(sb), "bc2": to_bf(bc2)}


def kernel(x, W_head, W_tail):
    x = np.asarray(x, np.float32)
    W_head = np.asarray(W_head, np.float32)
    W_tail = np.asarray(W_tail, np.float32)
    nc = _get_nc()
    consts = _const_maps(W_head, W_tail)
    in_maps = [{**m, **consts} for m in _host_prep(x)]
    res = run_bass_kernel_spmd(nc, in_maps, list(range(N_CORES)))
    out = np.empty_like(x)
    for core in range(N_CORES):
        b, half = core // 2, core % 2
        r0 = 64 * half
        y = res.results[core]["y"].astype(np.float32).reshape(128, ROUT, W)
        out[b, :, r0:r0 + 32, :] = y[:64]
        out[b, :, r0 + 32:r0 + 64, :] = y[64:]
    return out
